# revision 1
# baseline (speedup 1.0000x reference)
"""InterfaceBoundaryLoss Trainium2 kernel.

Data-parallel over batch across 8 NeuronCores.  The [H,W] interface mask is
analyzed on the host and covered with a small set of rectangular "boxes";
the device only streams / computes the boxed regions (the mask is a thin
circle, so this is ~4% of the dense frame).  Per box, all 8 local batches
are fused into the free dimension of [rows, 8*w] tiles.

Math (per batch b, cell (i,j) with mask m=1):
  pot += (phi1-phi2)^2
  der += (EPS1*d1 - EPS2*d2)^2,  dk = nx*dpx_k + ny*dpy_k
Let psi = 0.025*phi2 - phi1 = -(80*phi1 - 2*phi2)/80.  Then
  EPS1*d1 - EPS2*d2 = -40000*(nx*Dx(psi) + ny*Dy(psi))
with Dx/Dy the raw central differences.  So with host fields
  A = 40000*m*nx, B = 40000*m*ny      (zero off-mask)
  der = sum((A*Dx(psi) + B*Dy(psi))^2)
Dy is computed on the TensorEngine via a banded +/-1 stationary matrix,
Dx on the VectorEngine via shifted views.  Square+sum reductions run on
the ScalarEngine (activation Square with accum_out); the pot path runs
on GpSimd.  Host sums per-partition partials in float64.

Mask cells on the frame border (edge-padding semantics) are computed
exactly on the host (none for the reference circle mask).
"""

import sys

for _p in ("/opt/trn_rl_repo",):
    if _p not in sys.path:
        sys.path.append(_p)

import numpy as np
import ml_dtypes

B, H, W = 64, 1024, 1024
EPS1, EPS2 = 80.0, 2.0
DX, DY = 0.001, 0.001
CX, CY = 512.0, 512.0
WEIGHT = 1.0
N_CORES = 8
BPC = B // N_CORES

# "bf16" or "f32" device compute dtype.
DEVICE_DTYPE = "f32"

# set TRACE=True (e.g. from a test harness) to profile the device run;
# LAST_EXEC_NS then holds the measured NEFF execution time.
TRACE = False
LAST_EXEC_NS = None

_FD_CAP = {"f32": 2048, "bf16": 4096}


def _normals(h, w):
    ii = np.arange(h, dtype=np.float64)[:, None]
    jj = np.arange(w, dtype=np.float64)[None, :]
    nx = jj - CX
    ny = ii - CY
    norm = np.sqrt(nx * nx + ny * ny)
    safe = np.where(norm > 0, norm, 1.0)
    return nx / safe, ny / safe


def _cluster(cols, gap):
    """Group sorted col indices into (start, end) inclusive intervals."""
    out = []
    s = p = cols[0]
    for c in cols[1:]:
        if c - p > gap:
            out.append((s, p))
            s = c
        p = c
    out.append((s, p))
    return out


class _Box:
    __slots__ = ("r0", "nrows", "c0", "w", "nb", "ngroups")

    def __init__(self, r0, nrows, c0, w):
        self.r0, self.nrows, self.c0, self.w = int(r0), int(nrows), int(c0), int(w)


def _plan(mask):
    """Cover interior mask cells with boxes.

    Each box loads rows [r0, r0+nrows) x cols [c0, c0+w); cells assigned to
    it are in relative rows [1, nrows-2] and relative cols [1, w-2].
    Returns (boxes, assigned_masks, host_cells) where assigned_masks is the
    per-box bool array [nrows, w] of cells this box owns.
    """
    h, w_ = mask.shape
    border = np.zeros_like(mask)
    border[0, :] = border[-1, :] = True
    border[:, 0] = border[:, -1] = True
    host_cells = mask & border
    core = mask & ~border

    # Recursive cost-driven segmentation: a segment of rows is covered by
    # one box per column-cluster; split the segment in half whenever the
    # two halves' covers are cheaper (box fixed cost ~3000 cyc, ~22 cyc/col).
    def seg_specs(rs, h):
        cols = np.flatnonzero(core[rs : rs + h].any(axis=0))
        if len(cols) == 0:
            return 0.0, []
        clusters = _cluster(cols, gap=17)
        if len(clusters) > 2:
            clusters = [(cols[0], cols[-1])]
        cost = sum(3000.0 + 22.0 * (cb - ca + 10) for ca, cb in clusters)
        return cost, [(rs, h, clusters)]

    def dp(rs, h):
        c0, s0 = seg_specs(rs, h)
        if h <= 2 or not s0:
            return c0, s0
        h1 = h // 2
        ca_, sa = dp(rs, h1)
        cb_, sb = dp(rs + h1, h - h1)
        if ca_ + cb_ < c0:
            return ca_ + cb_, sa + sb
        return c0, s0

    boxes = []
    owned = []
    assigned = np.zeros_like(mask)
    rows_any = np.flatnonzero(core.any(axis=1))
    if len(rows_any):
        r = rows_any[0]
        rmax = rows_any[-1]
        specs = []
        while r <= rmax:
            if not core[r].any():
                r += 1
                continue
            h0 = min(126, rmax + 1 - r)
            _, s = dp(r, h0)
            specs.extend(s)
            r += h0
        for rs, hseg, clusters in specs:
            re_ = rs + hseg
            r = rs
            for ca0, cb0 in clusters:
                # split clusters wider than 498 so box width stays <= 512
                for ca in range(ca0, cb0 + 1, 498):
                    cb = min(ca + 497, cb0)
                    c0 = ca - 2
                    bw = cb + 3 - c0
                    if c0 % 2:
                        c0 -= 1
                        bw += 1
                    bw = -(-bw // 8) * 8
                    if c0 < 0:
                        c0 = 0
                    if c0 + bw > w_:
                        c0 = w_ - bw
                    bx = _Box(r - 1, hseg + 2, c0, bw)
                    sel = np.zeros((bx.nrows, bw), dtype=bool)
                    sub = core[r:re_, ca : cb + 1] & ~assigned[r:re_, ca : cb + 1]
                    sel[1 : 1 + hseg, ca - c0 : cb + 1 - c0] = sub
                    assigned[r:re_, ca : cb + 1] |= sub
                    boxes.append(bx)
                    owned.append(sel)
            r = re_

    leftover = core & ~assigned
    if leftover.any():
        host_cells = host_cells | leftover
        for sel, bx in zip(owned, boxes):
            lv = leftover[bx.r0 : bx.r0 + bx.nrows, bx.c0 : bx.c0 + bx.w]
            sel &= ~lv
    return boxes, owned, host_cells


def _host_contrib(cells_ij, phi1, phi2, nx, ny):
    """Exact (edge-padded) pot/der sums for the given cells, all batches."""
    if len(cells_ij[0]) == 0:
        return 0.0, 0.0
    ii, jj = cells_ij
    p1 = phi1.astype(np.float64)
    p2 = phi2.astype(np.float64)
    d = p1[:, ii, jj] - p2[:, ii, jj]
    pot = float(np.sum(d * d))

    # edge-padded central differences: clamp the *derivative* index
    jc = np.clip(jj, 1, W - 2)
    ic = np.clip(ii, 1, H - 2)

    def dn(p):
        dpx = (p[:, ii, jc + 1] - p[:, ii, jc - 1]) / (2.0 * DX)
        dpy = (p[:, ic + 1, jj] - p[:, ic - 1, jj]) / (2.0 * DY)
        return nx[ii, jj] * dpx + ny[ii, jj] * dpy

    mm = EPS1 * dn(p1) - EPS2 * dn(p2)
    der = float(np.sum(mm * mm))
    return pot, der


def _build_nc(boxes, dt_str, fd_cap):
    from contextlib import ExitStack
    from concourse import bass, bacc, tile, mybir

    mdt = mybir.dt.bfloat16 if dt_str == "bf16" else mybir.dt.float32
    f32 = mybir.dt.float32
    mult = mybir.AluOpType.mult
    sub = mybir.AluOpType.subtract
    SQ = mybir.ActivationFunctionType.Square

    njobs = sum(bx.ngroups for bx in boxes)
    nc = bacc.Bacc(
        "TRN2", target_bir_lowering=False, debug=False, num_devices=N_CORES
    )

    phi1_d = nc.dram_tensor("phi1", [BPC * H, W], mdt, kind="ExternalInput")
    phi2_d = nc.dram_tensor("phi2", [BPC * H, W], mdt, kind="ExternalInput")
    dmat_d = nc.dram_tensor("dmat", [128, 128], mdt, kind="ExternalInput")
    a_ds, b_ds, m_ds = [], [], []
    for k, bx in enumerate(boxes):
        a_ds.append(nc.dram_tensor(f"a{k}", [bx.nrows, bx.w], mdt, kind="ExternalInput"))
        b_ds.append(nc.dram_tensor(f"b{k}", [bx.nrows, bx.w], mdt, kind="ExternalInput"))
        m_ds.append(nc.dram_tensor(f"m{k}", [bx.nrows, bx.w], mdt, kind="ExternalInput"))
    acc_d = nc.dram_tensor("acc", [128, 2 * njobs], f32, kind="ExternalOutput")

    with tile.TileContext(nc) as tc, ExitStack() as ctx:
        io = ctx.enter_context(tc.tile_pool(name="io", bufs=3))
        cst = ctx.enter_context(tc.tile_pool(name="cst", bufs=2))
        mid = ctx.enter_context(tc.tile_pool(name="mid", bufs=2))
        pot_p = ctx.enter_context(tc.tile_pool(name="potp", bufs=2))
        psum = ctx.enter_context(tc.tile_pool(name="psum", bufs=4, space="PSUM"))
        onep = ctx.enter_context(tc.tile_pool(name="onep", bufs=1))

        dm = onep.tile([128, 128], mdt)
        nc.sync.dma_start(dm[:], dmat_d.ap())
        acc = onep.tile([128, 2 * njobs], f32)
        nc.vector.memset(acc[:], 0.0)

        job = 0
        for k, bx in enumerate(boxes):
            nr, w, nb = bx.nrows, bx.w, bx.nb
            fd = nb * w
            at = cst.tile([nr, w], mdt, tag="at")
            nc.sync.dma_start(at[:], a_ds[k].ap())
            bt = cst.tile([nr, w], mdt, tag="bt")
            nc.sync.dma_start(bt[:], b_ds[k].ap())
            mt = cst.tile([nr, w], mdt, tag="mt")
            nc.sync.dma_start(mt[:], m_ds[k].ap())
            a3 = at[:].unsqueeze(1).broadcast_to([nr, nb, w])
            b3 = bt[:].unsqueeze(1).broadcast_to([nr, nb, w])
            m3 = mt[:].unsqueeze(1).broadcast_to([nr, nb, w])

            for g in range(bx.ngroups):
                b0 = g * nb
                f1 = io.tile([nr, fd], mdt, tag="f1")
                f2 = io.tile([nr, fd], mdt, tag="f2")
                for ft, src_d in ((f1, phi1_d), (f2, phi2_d)):
                    src = bass.AP(
                        src_d,
                        (b0 * H + bx.r0) * W + bx.c0,
                        [[W, nr], [H * W, nb], [1, w]],
                    )
                    dst = ft[:].rearrange("p (b w) -> p b w", b=nb)
                    nc.sync.dma_start(dst, src)

                # psi = 0.025*phi2 - phi1
                psi = mid.tile([nr, fd], mdt, tag="psi")
                nc.vector.scalar_tensor_tensor(
                    psi[:], f2[:], 0.025, f1[:], op0=mult, op1=sub
                )

                # dxs[f] = psi[f+2] - psi[f]  (cell at f+1)
                dxs = mid.tile([nr, fd], mdt, tag="dxs")
                nc.vector.tensor_sub(
                    dxs[:, 0 : fd - 2], psi[:, 2:fd], psi[:, 0 : fd - 2]
                )
                nc.vector.memset(dxs[:, fd - 2 : fd], 0.0)
                u = mid.tile([nr, fd], mdt, tag="u")
                nc.vector.tensor_mul(
                    u[:].rearrange("p (b w) -> p b w", b=nb),
                    dxs[:].rearrange("p (b w) -> p b w", b=nb),
                    a3,
                )

                # dy via PE: dy[mi, f] = psi[mi+1, f] - psi[mi-1, f];
                # batch-aligned chunks of gchunk blocks (gchunk*w <= 512)
                v = mid.tile([nr, fd], mdt, tag="v")
                v3 = v[:].rearrange("p (b w) -> p b w", b=nb)
                nc.vector.memset(v3[:, :, w - 1 : w], 0.0)
                gchunk = max(1, 512 // w)
                for j0 in range(0, nb, gchunk):
                    gg = min(gchunk, nb - j0)
                    dy = psum.tile([128, 512], f32, tag="dy")
                    nc.tensor.matmul(
                        dy[:, 0 : gg * w],
                        dm[0:nr, :],
                        psi[:, j0 * w : (j0 + gg) * w],
                        start=True,
                        stop=True,
                    )
                    dy3 = dy[0:nr, 0 : gg * w].rearrange("p (b w) -> p b w", b=gg)
                    nc.vector.tensor_mul(
                        v3[:, j0 : j0 + gg, 0 : w - 1],
                        bt[:].unsqueeze(1).broadcast_to([nr, gg, w])[:, :, 0 : w - 1],
                        dy3[:, :, 1:w],
                    )

                wt = mid.tile([nr, fd], mdt, tag="wt")
                nc.vector.tensor_add(wt[:], u[:], v[:])
                nc.scalar.activation(
                    dxs[:],
                    wt[:],
                    SQ,
                    accum_out=acc[0:nr, njobs + job : njobs + job + 1],
                )

                # pot path on GpSimd
                df = pot_p.tile([nr, fd], mdt, tag="df")
                nc.gpsimd.tensor_sub(df[:], f2[:], f1[:])
                w1 = pot_p.tile([nr, fd], mdt, tag="w1")
                nc.gpsimd.tensor_mul(
                    w1[:].rearrange("p (b w) -> p b w", b=nb),
                    df[:].rearrange("p (b w) -> p b w", b=nb),
                    m3,
                )
                nc.scalar.activation(
                    df[:],
                    w1[:],
                    SQ,
                    accum_out=acc[0:nr, job : job + 1],
                )
                job += 1

        nc.sync.dma_start(acc_d.ap(), acc[:])

    nc.compile()
    return nc


def _prepare(mask):
    """Plan boxes and build all mask-derived constant arrays."""
    nx, ny = _normals(H, W)
    boxes, owned, host_cells = _plan(mask)

    fd_cap = _FD_CAP[DEVICE_DTYPE]
    np_dt = ml_dtypes.bfloat16 if DEVICE_DTYPE == "bf16" else np.float32

    for bx in boxes:
        nb = max(1, min(BPC, fd_cap // bx.w))
        while BPC % nb:
            nb -= 1
        bx.nb = nb
        bx.ngroups = BPC // nb

    consts = {}
    af = 40000.0 * nx
    bf = 40000.0 * ny
    for k, (bx, sel) in enumerate(zip(boxes, owned)):
        rs, cs = slice(bx.r0, bx.r0 + bx.nrows), slice(bx.c0, bx.c0 + bx.w)
        a_box = np.where(sel, af[rs, cs], 0.0)
        b_box = np.where(sel, bf[rs, cs], 0.0)
        # shift left by one col: field[k] = value at col k+1
        a_sh = np.zeros_like(a_box)
        a_sh[:, :-1] = a_box[:, 1:]
        b_sh = np.zeros_like(b_box)
        b_sh[:, :-1] = b_box[:, 1:]
        consts[f"a{k}"] = a_sh.astype(np_dt)
        consts[f"b{k}"] = b_sh.astype(np_dt)
        consts[f"m{k}"] = sel.astype(np_dt)

    dmat = np.zeros((128, 128), dtype=np.float64)
    for mi in range(1, 127):
        dmat[mi + 1, mi] = 1.0
        dmat[mi - 1, mi] = -1.0
    consts["dmat"] = dmat.astype(np_dt)
    return boxes, consts, host_cells, np_dt


_CACHE = {}


def kernel(output_in, output_out, interface_mask):
    from concourse.bass_utils import run_bass_kernel_spmd

    phi1 = np.asarray(output_in).reshape(B, H, W)
    phi2 = np.asarray(output_out).reshape(B, H, W)
    mask = np.asarray(interface_mask).astype(bool)

    n_mask = float(mask.sum())
    if n_mask == 0.0:
        return np.float32(np.nan)

    key = (mask.tobytes(), DEVICE_DTYPE)
    if key not in _CACHE:
        boxes, consts, host_cells, np_dt = _prepare(mask)
        nc = _build_nc(boxes, DEVICE_DTYPE, _FD_CAP[DEVICE_DTYPE]) if boxes else None
        _CACHE[key] = (boxes, consts, host_cells, np_dt, nc)
    boxes, consts, host_cells, np_dt, nc = _CACHE[key]

    pot = der = 0.0
    if nc is not None:
        in_maps = []
        for c in range(N_CORES):
            sl = slice(c * BPC, (c + 1) * BPC)
            m = dict(consts)
            m["phi1"] = np.ascontiguousarray(phi1[sl]).reshape(BPC * H, W).astype(np_dt)
            m["phi2"] = np.ascontiguousarray(phi2[sl]).reshape(BPC * H, W).astype(np_dt)
            in_maps.append(m)
        res = run_bass_kernel_spmd(
            nc, in_maps, core_ids=list(range(N_CORES)), trace=TRACE
        )
        global LAST_EXEC_NS
        LAST_EXEC_NS = res.exec_time_ns
        njobs = sum(bx.ngroups for bx in boxes)
        for r in res.results:
            a = r["acc"].astype(np.float64)
            pot += float(a[:, :njobs].sum())
            der += float(a[:, njobs:].sum())

    if host_cells.any():
        nx, ny = _normals(H, W)
        hp, hd = _host_contrib(np.nonzero(host_cells), phi1, phi2, nx, ny)
        pot += hp
        der += hd

    denom = B * n_mask
    return np.float32(WEIGHT * (pot + der) / denom)



# revision 5
# speedup vs baseline: 1.6692x; 1.6692x over previous
"""InterfaceBoundaryLoss Trainium2 kernel.

Data-parallel over batch across 8 NeuronCores.  The [H,W] interface mask is
analyzed on the host and covered with variable-height "segments" (tall thin
ones along the near-vertical arcs, short wide ones near the circle's
top/bottom vertices).  Segments are packed into the 128 SBUF partitions in
"groups" sharing a uniform column width, so every engine instruction runs
at full partition occupancy while the free dim stays small.

Per masked cell (i,j) with m=1:
  pot += (phi1-phi2)^2
  der += (EPS1*d1 - EPS2*d2)^2,  dk = nx*dpx_k + ny*dpy_k
With psi = 0.025*phi2 - phi1,  EPS1*d1 - EPS2*d2 = -40000*(nx*Dx(psi) +
ny*Dy(psi)), so with host fields A = 40000*m*nx, B = 40000*m*ny,
  der = sum((A*Dx(psi) + B*Dy(psi))^2)
Dy is computed on the TensorEngine via a per-group block-banded stationary
matrix (one +/-1 band per packed segment), Dx on the VectorEngine via
shifted views.  The pot path runs on GpSimd.  Both quadratic terms are
packed side by side in one tile and reduced with a single Square+accum on
the ScalarEngine per group.

phi1/phi2 are interleaved on the host as [BPC, 2, H, W] so one 3D DMA per
segment fetches all 8 local batches of both fields (the batch*field axis is
a single uniform-stride dim).  DMA dispatch serializes on the shared HWDGE
unit (~630ns each), so the planner trades segment count against free-dim
size explicitly.

Host sums per-partition partials in float64; mask cells on the frame border
(edge-padding semantics) or uncovered by segments are computed exactly on
the host (none for the reference circle mask).
"""

import sys

for _p in ("/opt/trn_rl_repo",):
    if _p not in sys.path:
        sys.path.append(_p)

import numpy as np
import ml_dtypes

B, H, W = 64, 1024, 1024
EPS1, EPS2 = 80.0, 2.0
DX, DY = 0.001, 0.001
CX, CY = 512.0, 512.0
WEIGHT = 1.0
N_CORES = 8
BPC = B // N_CORES
NBF = 2 * BPC  # batch*field blocks per partition row

# planner cost model: ns per DMA dispatch vs ns per free-dim column
PLAN_DISP = 1260.0
PLAN_ENG = 6.1
CHUNK_W = 59  # max owned cols per segment -> w_g <= 64, fd <= 512
HEIGHTS = (126, 62, 30, 14)
CLUSTER_GAP = 8

TRACE = False
LAST_EXEC_NS = None


def _normals(h, w):
    ii = np.arange(h, dtype=np.float64)[:, None]
    jj = np.arange(w, dtype=np.float64)[None, :]
    nx = jj - CX
    ny = ii - CY
    norm = np.sqrt(nx * nx + ny * ny)
    safe = np.where(norm > 0, norm, 1.0)
    return nx / safe, ny / safe


def _cluster(cols, gap):
    out = []
    s = p = cols[0]
    for c in cols[1:]:
        if c - p > gap:
            out.append((s, p))
            s = c
        p = c
    out.append((s, p))
    return out


class _Seg:
    __slots__ = ("r0", "h", "ca", "ce", "c0", "p0", "owned")

    def __init__(self, r0, h, ca, ce):
        self.r0, self.h, self.ca, self.ce = int(r0), int(h), int(ca), int(ce)

    @property
    def prows(self):
        return self.h + 2


class _Group:
    def __init__(self):
        self.segs = []
        self._p = 0
        self.w = 0
        self.coff = 0


def _plan(mask):
    """Cover interior mask cells with variable-height segments, pack into
    128-partition groups of uniform width."""
    h_, w_ = mask.shape
    border = np.zeros_like(mask)
    border[0, :] = border[-1, :] = True
    border[:, 0] = border[:, -1] = True
    host_cells = mask & border
    core = mask & ~border

    rows_any = np.flatnonzero(core.any(axis=1))
    if len(rows_any) == 0:
        return [], host_cells

    minr, maxr = int(rows_any[0]), int(rows_any[-1])

    def band_segments(r0, hh):
        band = core[r0 : r0 + hh]
        cols = np.flatnonzero(band.any(axis=0))
        if len(cols) == 0:
            return []
        segs = []
        for ca, cb in _cluster(cols, CLUSTER_GAP):
            n = cb - ca + 1
            nch = -(-n // CHUNK_W)
            step = -(-n // nch)
            for k in range(nch):
                a = ca + k * step
                e = min(ca + (k + 1) * step - 1, cb)
                segs.append(_Seg(r0, hh, a, e))
        return segs

    def seg_cost(s):
        w = -(-(s.ce - s.ca + 1 + 5) // 8) * 8
        return PLAN_DISP + PLAN_ENG * NBF // 2 * w * (s.h + 2) / 128.0

    memo = {}

    def dp(r):
        if r > maxr:
            return (0.0, ())
        if r in memo:
            return memo[r]
        best = None
        for hh in HEIGHTS:
            he = min(hh, maxr + 1 - r)
            segs = band_segments(r, he)
            c = sum(seg_cost(s) for s in segs)
            sub, subsegs = dp(r + he)
            tot = c + sub
            if best is None or tot < best[0]:
                best = (tot, tuple(segs) + subsegs)
        memo[r] = best
        return best

    _, segs = dp(minr)
    segs = list(segs)

    # pack into groups: widest first, greedy partition fill
    segs.sort(key=lambda s: -(s.ce - s.ca))
    groups = []
    for s in segs:
        for g in groups:
            if g._p + s.prows <= 128:
                g.segs.append(s)
                g._p += s.prows
                break
        else:
            g = _Group()
            g._p = s.prows
            g.segs.append(s)
            groups.append(g)

    # per group: uniform width, per-seg c0/p0, owned cells (dedup)
    assigned = np.zeros_like(mask)
    for g in groups:
        w = max(-(-(s.ce - s.ca + 1 + 5) // 8) * 8 for s in g.segs)
        g.w = w
        p0 = 0
        for s in g.segs:
            c0 = s.ca - 2
            if c0 % 2:
                c0 -= 1
            c0 = max(0, min(c0, w_ - w))
            s.c0 = c0
            s.p0 = p0
            p0 += s.prows
            ce = min(s.ce, c0 + w - 3)  # keep owned >=2 cols from window edge
            own = np.zeros((s.prows, w), dtype=bool)
            sub = (
                core[s.r0 : s.r0 + s.h, s.ca : ce + 1]
                & ~assigned[s.r0 : s.r0 + s.h, s.ca : ce + 1]
            )
            own[1 : 1 + s.h, s.ca - c0 : ce + 1 - c0] = sub
            assigned[s.r0 : s.r0 + s.h, s.ca : ce + 1] |= sub
            s.owned = own

    leftover = core & ~assigned
    if leftover.any():
        host_cells = host_cells | leftover
        for g in groups:
            for s in g.segs:
                lv = leftover[s.r0 - 1 : s.r0 - 1 + s.prows, s.c0 : s.c0 + g.w]
                s.owned &= ~lv
    return groups, host_cells


def _host_contrib(cells_ij, phi1, phi2, nx, ny):
    if len(cells_ij[0]) == 0:
        return 0.0, 0.0
    ii, jj = cells_ij
    p1 = phi1.astype(np.float64)
    p2 = phi2.astype(np.float64)
    d = p1[:, ii, jj] - p2[:, ii, jj]
    pot = float(np.sum(d * d))
    jc = np.clip(jj, 1, W - 2)
    ic = np.clip(ii, 1, H - 2)

    def dn(p):
        dpx = (p[:, ii, jc + 1] - p[:, ii, jc - 1]) / (2.0 * DX)
        dpy = (p[:, ic + 1, jj] - p[:, ic - 1, jj]) / (2.0 * DY)
        return nx[ii, jj] * dpx + ny[ii, jj] * dpy

    mm = EPS1 * dn(p1) - EPS2 * dn(p2)
    der = float(np.sum(mm * mm))
    return pot, der


def _prepare(mask):
    nx, ny = _normals(H, W)
    groups, host_cells = _plan(mask)
    np_dt = ml_dtypes.bfloat16

    af = 40000.0 * nx
    bf = 40000.0 * ny

    ng = len(groups)
    ctot = sum(3 * g.w for g in groups)
    cst = np.zeros((128, ctot), dtype=np.float64)
    dmats = np.zeros((128, 128 * ng), dtype=np.float64)
    coff = 0
    for gi, g in enumerate(groups):
        g.coff = coff
        w = g.w
        for s in g.segs:
            rs = slice(s.r0 - 1, s.r0 - 1 + s.prows)
            cs = slice(s.c0, s.c0 + w)
            a_box = np.where(s.owned, af[rs, cs], 0.0)
            b_box = np.where(s.owned, bf[rs, cs], 0.0)
            ps = slice(s.p0, s.p0 + s.prows)
            # pre-shift A/B left one col: field[f] = value at col f+1
            cst[ps, coff : coff + w - 1] = a_box[:, 1:]
            cst[ps, coff + w : coff + 2 * w - 1] = b_box[:, 1:]
            cst[ps, coff + 2 * w : coff + 3 * w] = s.owned
            # dmat block: dy[p] = psi[p+1] - psi[p-1] on interior rows
            for lr in range(1, s.h + 1):
                p = s.p0 + lr
                dmats[p + 1, gi * 128 + p] = 1.0
                dmats[p - 1, gi * 128 + p] = -1.0
        coff += 3 * w

    consts = {
        "cst": cst.astype(np_dt),
        "dmats": dmats.astype(np_dt),
    }
    return groups, consts, host_cells, np_dt


def _build_nc(groups, ctot):
    from contextlib import ExitStack
    from concourse import bass, bacc, tile, mybir

    mdt = mybir.dt.bfloat16
    f32 = mybir.dt.float32
    mult = mybir.AluOpType.mult
    sub = mybir.AluOpType.subtract
    SQ = mybir.ActivationFunctionType.Square

    ng = len(groups)
    nc = bacc.Bacc(
        "TRN2", target_bir_lowering=False, debug=False, num_devices=N_CORES
    )

    pf_d = nc.dram_tensor("pf", [BPC * 2 * H, W], mdt, kind="ExternalInput")
    cst_d = nc.dram_tensor("cst", [128, ctot], mdt, kind="ExternalInput")
    dm_d = nc.dram_tensor("dmats", [128, 128 * ng], mdt, kind="ExternalInput")
    acc_d = nc.dram_tensor("acc", [128, ng], f32, kind="ExternalOutput")

    with tile.TileContext(nc) as tc, ExitStack() as ctx:
        io = ctx.enter_context(tc.tile_pool(name="io", bufs=3))
        mid = ctx.enter_context(tc.tile_pool(name="mid", bufs=2))
        pot_p = ctx.enter_context(tc.tile_pool(name="potp", bufs=2))
        psum = ctx.enter_context(tc.tile_pool(name="psum", bufs=4, space="PSUM"))
        onep = ctx.enter_context(tc.tile_pool(name="onep", bufs=1))

        cstt = onep.tile([128, ctot], mdt)
        nc.scalar.dma_start(cstt[:], cst_d.ap())
        dmt = onep.tile([128, 128 * ng], mdt)
        nc.scalar.dma_start(dmt[:], dm_d.ap())
        acc = onep.tile([128, ng], f32)
        nc.vector.memset(acc[:], 0.0)

        for gi, g in enumerate(groups):
            w = g.w
            fd = BPC * w
            P = sum(s.prows for s in g.segs)
            co = g.coff

            ft = io.tile([128, NBF * w], mdt, tag="ft")
            for s in g.segs:
                src = bass.AP(
                    pf_d,
                    (s.r0 - 1) * W + s.c0,
                    [[W, s.prows], [H * W, NBF], [1, w]],
                )
                dst = ft[s.p0 : s.p0 + s.prows, :].rearrange(
                    "p (q w) -> p q w", q=NBF
                )
                nc.sync.dma_start(dst, src)

            ft4 = ft[0:P].rearrange("p (b f w) -> p b f w", b=BPC, f=2)
            f1v = ft4[:, :, 0, :]
            f2v = ft4[:, :, 1, :]

            # psi = 0.025*phi2 - phi1
            psi = mid.tile([128, fd], mdt, tag="psi")
            psi3 = psi[0:P].rearrange("p (b w) -> p b w", b=BPC)
            nc.vector.scalar_tensor_tensor(
                psi3, f2v, 0.025, f1v, op0=mult, op1=sub
            )

            # dxs[f] = psi[f+2] - psi[f]  (cell at f+1)
            dxs = mid.tile([128, fd], mdt, tag="dxs")
            nc.vector.tensor_sub(
                dxs[0:P, 0 : fd - 2], psi[0:P, 2:fd], psi[0:P, 0 : fd - 2]
            )
            nc.vector.memset(dxs[0:P, fd - 2 : fd], 0.0)

            a3 = cstt[0:P, co : co + w].unsqueeze(1).broadcast_to([P, BPC, w])
            u = mid.tile([128, fd], mdt, tag="u")
            nc.vector.tensor_mul(
                u[0:P].rearrange("p (b w) -> p b w", b=BPC),
                dxs[0:P].rearrange("p (b w) -> p b w", b=BPC),
                a3,
            )

            # dy via PE: block-banded stationary per group
            dy = psum.tile([128, 512], f32, tag="dy")
            nc.tensor.matmul(
                dy[:, 0:fd],
                dmt[0:P, gi * 128 : gi * 128 + 128],
                psi[0:P, 0:fd],
                start=True,
                stop=True,
            )
            dy3 = dy[0:P, 0:fd].rearrange("p (b w) -> p b w", b=BPC)
            b3 = (
                cstt[0:P, co + w : co + 2 * w]
                .unsqueeze(1)
                .broadcast_to([P, BPC, w])
            )
            v = mid.tile([128, fd], mdt, tag="v")
            v3 = v[0:P].rearrange("p (b w) -> p b w", b=BPC)
            nc.vector.memset(v3[:, :, w - 1 : w], 0.0)
            nc.vector.tensor_mul(
                v3[:, :, 0 : w - 1], b3[:, :, 0 : w - 1], dy3[:, :, 1:w]
            )

            z = mid.tile([128, 2 * fd], mdt, tag="z")
            nc.vector.tensor_add(z[0:P, 0:fd], u[0:P, :], v[0:P, :])

            # pot path on GpSimd
            df = pot_p.tile([128, fd], mdt, tag="df")
            df3 = df[0:P].rearrange("p (b w) -> p b w", b=BPC)
            nc.gpsimd.tensor_sub(df3, f2v, f1v)
            m3 = (
                cstt[0:P, co + 2 * w : co + 3 * w]
                .unsqueeze(1)
                .broadcast_to([P, BPC, w])
            )
            nc.gpsimd.tensor_mul(
                z[0:P, fd : 2 * fd].rearrange("p (b w) -> p b w", b=BPC),
                df3,
                m3,
            )

            zsq = mid.tile([128, 2 * fd], mdt, tag="zsq")
            nc.scalar.activation(
                zsq[0:P, :],
                z[0:P, :],
                SQ,
                accum_out=acc[0:P, gi : gi + 1],
            )

        nc.sync.dma_start(acc_d.ap(), acc[:])

    nc.compile()
    return nc


_CACHE = {}


def kernel(output_in, output_out, interface_mask):
    from concourse.bass_utils import run_bass_kernel_spmd

    phi1 = np.asarray(output_in).reshape(B, H, W)
    phi2 = np.asarray(output_out).reshape(B, H, W)
    mask = np.asarray(interface_mask).astype(bool)

    n_mask = float(mask.sum())
    if n_mask == 0.0:
        return np.float32(np.nan)

    key = mask.tobytes()
    if key not in _CACHE:
        groups, consts, host_cells, np_dt = _prepare(mask)
        ctot = sum(3 * g.w for g in groups)
        nc = _build_nc(groups, ctot) if groups else None
        _CACHE[key] = (groups, consts, host_cells, np_dt, nc)
    groups, consts, host_cells, np_dt, nc = _CACHE[key]

    pot = der = 0.0
    if nc is not None:
        in_maps = []
        for c in range(N_CORES):
            sl = slice(c * BPC, (c + 1) * BPC)
            m = dict(consts)
            pf = np.stack(
                [phi1[sl], phi2[sl]], axis=1
            )  # [BPC, 2, H, W]
            m["pf"] = pf.reshape(BPC * 2 * H, W).astype(np_dt)
            in_maps.append(m)
        res = run_bass_kernel_spmd(
            nc, in_maps, core_ids=list(range(N_CORES)), trace=TRACE
        )
        global LAST_EXEC_NS
        LAST_EXEC_NS = res.exec_time_ns
        for r in res.results:
            a = r["acc"].astype(np.float64)
            both = float(a.sum())
            pot += 0.0
            der += both  # pot+der combined in one accumulator

    if host_cells.any():
        nx, ny = _normals(H, W)
        hp, hd = _host_contrib(np.nonzero(host_cells), phi1, phi2, nx, ny)
        pot += hp
        der += hd

    denom = B * n_mask
    return np.float32(WEIGHT * (pot + der) / denom)


# revision 12
# speedup vs baseline: 2.0086x; 1.2033x over previous
"""InterfaceBoundaryLoss Trainium2 kernel.

Data-parallel over batch across 8 NeuronCores.  The [H,W] interface mask is
analyzed on the host and covered with variable-height "segments" (tall thin
ones along the near-vertical arcs, short wide ones near the circle's
top/bottom vertices).  Segments are packed into the 128 SBUF partitions in
"groups" sharing a uniform column width, so every engine instruction runs
at full partition occupancy while the free dim stays small.

Per masked cell (i,j) with m=1:
  pot += (phi1-phi2)^2
  der += (EPS1*d1 - EPS2*d2)^2,  dk = nx*dpx_k + ny*dpy_k
With psi = 0.025*phi2 - phi1,  EPS1*d1 - EPS2*d2 = -40000*(nx*Dx(psi) +
ny*Dy(psi)), so with host fields A = 40000*m*nx, B = 40000*m*ny,
  der = sum((A*Dx(psi) + B*Dy(psi))^2)
Dy is computed on the TensorEngine via a per-group block-banded stationary
matrix (one +/-1 band per packed segment), Dx on the VectorEngine via
shifted views.  The pot path runs on GpSimd.  Both quadratic terms are
packed side by side in one tile and reduced with a single Square+accum on
the ScalarEngine per group.

phi1/phi2 are interleaved on the host as [BPC, 2, H, W] so one 3D DMA per
segment fetches all 8 local batches of both fields (the batch*field axis is
a single uniform-stride dim).  DMA dispatch serializes on the shared HWDGE
unit (~630ns each), so the planner trades segment count against free-dim
size explicitly.

Host sums per-partition partials in float64; mask cells on the frame border
(edge-padding semantics) or uncovered by segments are computed exactly on
the host (none for the reference circle mask).
"""

import sys

for _p in ("/opt/trn_rl_repo",):
    if _p not in sys.path:
        sys.path.append(_p)

import numpy as np
import ml_dtypes

B, H, W = 64, 1024, 1024
EPS1, EPS2 = 80.0, 2.0
DX, DY = 0.001, 0.001
CX, CY = 512.0, 512.0
WEIGHT = 1.0
N_CORES = 8
BPC = B // N_CORES
NBF = 2 * BPC  # batch*field blocks per partition row

# planner cost model: ns per DMA dispatch vs ns per free-dim column
PLAN_DISP = 1260.0
PLAN_ENG = 6.1
CHUNK_W = 59  # max owned cols per segment -> w_g <= 64, fd <= 512
HEIGHTS = (126, 62, 30, 14)
CLUSTER_GAP = 8

TRACE = False
LAST_EXEC_NS = None


def _normals(h, w):
    ii = np.arange(h, dtype=np.float64)[:, None]
    jj = np.arange(w, dtype=np.float64)[None, :]
    nx = jj - CX
    ny = ii - CY
    norm = np.sqrt(nx * nx + ny * ny)
    safe = np.where(norm > 0, norm, 1.0)
    return nx / safe, ny / safe


def _cluster(cols, gap):
    out = []
    s = p = cols[0]
    for c in cols[1:]:
        if c - p > gap:
            out.append((s, p))
            s = c
        p = c
    out.append((s, p))
    return out


class _Seg:
    __slots__ = ("r0", "h", "ca", "ce", "c0", "p0", "owned")

    def __init__(self, r0, h, ca, ce):
        self.r0, self.h, self.ca, self.ce = int(r0), int(h), int(ca), int(ce)

    @property
    def prows(self):
        return self.h + 2


class _Group:
    def __init__(self):
        self.segs = []
        self._p = 0
        self.w = 0
        self.coff = 0


def _plan(mask):
    """Cover interior mask cells with variable-height segments, pack into
    128-partition groups of uniform width."""
    h_, w_ = mask.shape
    border = np.zeros_like(mask)
    border[0, :] = border[-1, :] = True
    border[:, 0] = border[:, -1] = True
    host_cells = mask & border
    core = mask & ~border

    rows_any = np.flatnonzero(core.any(axis=1))
    if len(rows_any) == 0:
        return [], host_cells

    minr, maxr = int(rows_any[0]), int(rows_any[-1])

    def band_segments(r0, hh):
        band = core[r0 : r0 + hh]
        cols = np.flatnonzero(band.any(axis=0))
        if len(cols) == 0:
            return []
        segs = []
        for ca, cb in _cluster(cols, CLUSTER_GAP):
            n = cb - ca + 1
            nch = -(-n // CHUNK_W)
            step = -(-n // nch)
            for k in range(nch):
                a = ca + k * step
                e = min(ca + (k + 1) * step - 1, cb)
                segs.append(_Seg(r0, hh, a, e))
        return segs

    def seg_cost(s):
        w = -(-(s.ce - s.ca + 1 + 5) // 8) * 8
        return PLAN_DISP + PLAN_ENG * NBF // 2 * w * (s.h + 2) / 128.0

    memo = {}

    def dp(r):
        if r > maxr:
            return (0.0, ())
        if r in memo:
            return memo[r]
        best = None
        for hh in HEIGHTS:
            he = min(hh, maxr + 1 - r)
            segs = band_segments(r, he)
            c = sum(seg_cost(s) for s in segs)
            sub, subsegs = dp(r + he)
            tot = c + sub
            if best is None or tot < best[0]:
                best = (tot, tuple(segs) + subsegs)
        memo[r] = best
        return best

    _, segs = dp(minr)
    segs = list(segs)

    # pack into groups: widest first, greedy partition fill
    segs.sort(key=lambda s: -(s.ce - s.ca))
    groups = []
    for s in segs:
        for g in groups:
            if g._p + s.prows <= 128:
                g.segs.append(s)
                g._p += s.prows
                break
        else:
            g = _Group()
            g._p = s.prows
            g.segs.append(s)
            groups.append(g)

    # per group: uniform width, per-seg c0/p0, owned cells (dedup)
    assigned = np.zeros_like(mask)
    for g in groups:
        w = max(-(-(s.ce - s.ca + 1 + 5) // 8) * 8 for s in g.segs)
        g.w = w
        p0 = 0
        for s in g.segs:
            c0 = s.ca - 2
            if c0 % 2:
                c0 -= 1
            c0 = max(0, min(c0, w_ - w))
            s.c0 = c0
            s.p0 = p0
            p0 += s.prows
            ce = min(s.ce, c0 + w - 3)  # keep owned >=2 cols from window edge
            own = np.zeros((s.prows, w), dtype=bool)
            sub = (
                core[s.r0 : s.r0 + s.h, s.ca : ce + 1]
                & ~assigned[s.r0 : s.r0 + s.h, s.ca : ce + 1]
            )
            own[1 : 1 + s.h, s.ca - c0 : ce + 1 - c0] = sub
            assigned[s.r0 : s.r0 + s.h, s.ca : ce + 1] |= sub
            s.owned = own

    leftover = core & ~assigned
    if leftover.any():
        host_cells = host_cells | leftover
        for g in groups:
            for s in g.segs:
                lv = leftover[s.r0 - 1 : s.r0 - 1 + s.prows, s.c0 : s.c0 + g.w]
                s.owned &= ~lv
    return groups, host_cells


def _host_contrib(cells_ij, phi1, phi2, nx, ny):
    if len(cells_ij[0]) == 0:
        return 0.0, 0.0
    ii, jj = cells_ij
    p1 = phi1.astype(np.float64)
    p2 = phi2.astype(np.float64)
    d = p1[:, ii, jj] - p2[:, ii, jj]
    pot = float(np.sum(d * d))
    jc = np.clip(jj, 1, W - 2)
    ic = np.clip(ii, 1, H - 2)

    def dn(p):
        dpx = (p[:, ii, jc + 1] - p[:, ii, jc - 1]) / (2.0 * DX)
        dpy = (p[:, ic + 1, jj] - p[:, ic - 1, jj]) / (2.0 * DY)
        return nx[ii, jj] * dpx + ny[ii, jj] * dpy

    mm = EPS1 * dn(p1) - EPS2 * dn(p2)
    der = float(np.sum(mm * mm))
    return pot, der


def _prepare(mask):
    nx, ny = _normals(H, W)
    groups, host_cells = _plan(mask)
    np_dt = ml_dtypes.bfloat16

    af = 40000.0 * nx
    bf = 40000.0 * ny

    ng = len(groups)
    ctot = sum(3 * g.w for g in groups)
    cst = np.zeros((128, ctot), dtype=np.float64)
    # blocks 0..ng-1: per-group banded Dy matrices; block ng: -I; ng+1: +I
    dmats = np.zeros((128, 128 * (ng + 2)), dtype=np.float64)
    for p in range(128):
        dmats[p, 128 * ng + p] = -1.0
        dmats[p, 128 * (ng + 1) + p] = 1.0
    coff = 0
    for gi, g in enumerate(groups):
        g.coff = coff
        w = g.w
        for s in g.segs:
            rs = slice(s.r0 - 1, s.r0 - 1 + s.prows)
            cs = slice(s.c0, s.c0 + w)
            a_box = np.where(s.owned, af[rs, cs], 0.0)
            b_box = np.where(s.owned, bf[rs, cs], 0.0)
            ps = slice(s.p0, s.p0 + s.prows)
            # pre-shift A/B left one col: field[f] = value at col f+1
            cst[ps, coff : coff + w - 1] = a_box[:, 1:]
            cst[ps, coff + w : coff + 2 * w - 1] = b_box[:, 1:]
            cst[ps, coff + 2 * w : coff + 3 * w] = s.owned
            # dmat block: dy[p] = psi[p+1] - psi[p-1] on interior rows
            for lr in range(1, s.h + 1):
                p = s.p0 + lr
                dmats[p + 1, gi * 128 + p] = 1.0
                dmats[p - 1, gi * 128 + p] = -1.0
        coff += 3 * w

    consts = {
        "cst": cst.astype(np_dt),
        "dmats": dmats.astype(np_dt),
    }
    return groups, consts, host_cells, np_dt


def _build_nc(groups, ctot):
    from contextlib import ExitStack
    from concourse import bass, bacc, tile, mybir

    mdt = mybir.dt.bfloat16
    f32 = mybir.dt.float32
    mult = mybir.AluOpType.mult
    sub = mybir.AluOpType.subtract
    SQ = mybir.ActivationFunctionType.Square

    ng = len(groups)
    nc = bacc.Bacc(
        "TRN2", target_bir_lowering=False, debug=False, num_devices=N_CORES
    )

    pf_d = nc.dram_tensor("pf", [2 * BPC * H, W], mdt, kind="ExternalInput")
    cst_d = nc.dram_tensor("cst", [128, ctot], mdt, kind="ExternalInput")
    dm_d = nc.dram_tensor(
        "dmats", [128, 128 * (ng + 2)], mdt, kind="ExternalInput"
    )
    acc_d = nc.dram_tensor("acc", [128, ng], f32, kind="ExternalOutput")

    with tile.TileContext(nc) as tc, ExitStack() as ctx:
        io = ctx.enter_context(tc.tile_pool(name="io", bufs=4))
        mid = ctx.enter_context(tc.tile_pool(name="mid", bufs=3))
        psum = ctx.enter_context(tc.tile_pool(name="psum", bufs=3, space="PSUM"))
        onep = ctx.enter_context(tc.tile_pool(name="onep", bufs=1))

        cstt = onep.tile([128, ctot], mdt)
        nc.scalar.dma_start(cstt[:], cst_d.ap())
        dmt = onep.tile([128, 128 * (ng + 2)], mdt)
        nc.scalar.dma_start(dmt[:], dm_d.ap())
        acc = onep.tile([128, ng], f32)
        nc.vector.memset(acc[:], 0.0)

        for gi, g in enumerate(groups):
            w = g.w
            fd = BPC * w
            P = sum(s.prows for s in g.segs)
            co = g.coff

            ft = io.tile([128, NBF * w], mdt, tag="ft")
            for s in g.segs:
                src = bass.AP(
                    pf_d,
                    (s.r0 - 1) * W + s.c0,
                    [[W, s.prows], [H * W, NBF], [1, w]],
                )
                dst = ft[s.p0 : s.p0 + s.prows, :].rearrange(
                    "p (q w) -> p q w", q=NBF
                )
                nc.sync.dma_start(dst, src)

            f1v = ft[0:P, 0:fd]
            f2v = ft[0:P, fd : 2 * fd]

            # psi = 0.025*phi2 - phi1
            psi = mid.tile([128, fd], mdt, tag="psi")
            nc.vector.scalar_tensor_tensor(
                psi[0:P, :], f2v, 0.025, f1v, op0=mult, op1=sub
            )

            # dxs[f] = psi[f+2] - psi[f]  (cell at f+1)
            dxs = mid.tile([128, fd], mdt, tag="dxs")
            nc.vector.tensor_sub(
                dxs[0:P, 0 : fd - 2], psi[0:P, 2:fd], psi[0:P, 0 : fd - 2]
            )
            nc.vector.memset(dxs[0:P, fd - 2 : fd], 0.0)

            a3 = cstt[0:P, co : co + w].unsqueeze(1).broadcast_to([P, BPC, w])
            u = mid.tile([128, fd], mdt, tag="u")
            nc.vector.tensor_mul(
                u[0:P].rearrange("p (b w) -> p b w", b=BPC),
                dxs[0:P].rearrange("p (b w) -> p b w", b=BPC),
                a3,
            )

            # dy via PE: block-banded stationary per group
            dy = psum.tile([128, 512], f32, tag="dy")
            nc.tensor.matmul(
                dy[:, 0:fd],
                dmt[0:P, gi * 128 : gi * 128 + 128],
                psi[0:P, 0:fd],
                start=True,
                stop=True,
            )
            dy3 = dy[0:P, 0:fd].rearrange("p (b w) -> p b w", b=BPC)
            b3 = (
                cstt[0:P, co + w : co + 2 * w]
                .unsqueeze(1)
                .broadcast_to([P, BPC, w])
            )
            v = mid.tile([128, fd], mdt, tag="v")
            v3 = v[0:P].rearrange("p (b w) -> p b w", b=BPC)
            nc.vector.memset(v3[:, :, w - 1 : w], 0.0)
            nc.vector.tensor_mul(
                v3[:, :, 0 : w - 1], b3[:, :, 0 : w - 1], dy3[:, :, 1:w]
            )

            z = mid.tile([128, 2 * fd], mdt, tag="z")
            nc.vector.tensor_add(z[0:P, 0:fd], u[0:P, :], v[0:P, :])

            # pot path on GpSimd (2D contiguous field slices)
            df = mid.tile([128, fd], mdt, tag="df")
            nc.gpsimd.tensor_sub(df[0:P, :], f2v, f1v)
            m3 = (
                cstt[0:P, co + 2 * w : co + 3 * w]
                .unsqueeze(1)
                .broadcast_to([P, BPC, w])
            )
            nc.gpsimd.tensor_mul(
                z[0:P, fd : 2 * fd].rearrange("p (b w) -> p b w", b=BPC),
                df[0:P].rearrange("p (b w) -> p b w", b=BPC),
                m3,
            )

            zsq = mid.tile([128, 2 * fd], mdt, tag="zsq")
            nc.scalar.activation(
                zsq[0:P, :],
                z[0:P, :],
                SQ,
                accum_out=acc[0:P, gi : gi + 1],
            )

        nc.sync.dma_start(acc_d.ap(), acc[:])

    nc.compile()
    return nc


_CACHE = {}


def kernel(output_in, output_out, interface_mask):
    from concourse.bass_utils import run_bass_kernel_spmd

    phi1 = np.asarray(output_in).reshape(B, H, W)
    phi2 = np.asarray(output_out).reshape(B, H, W)
    mask = np.asarray(interface_mask).astype(bool)

    n_mask = float(mask.sum())
    if n_mask == 0.0:
        return np.float32(np.nan)

    key = mask.tobytes()
    if key not in _CACHE:
        groups, consts, host_cells, np_dt = _prepare(mask)
        ctot = sum(3 * g.w for g in groups)
        nc = _build_nc(groups, ctot) if groups else None
        _CACHE[key] = (groups, consts, host_cells, np_dt, nc)
    groups, consts, host_cells, np_dt, nc = _CACHE[key]

    pot = der = 0.0
    if nc is not None:
        in_maps = []
        for c in range(N_CORES):
            sl = slice(c * BPC, (c + 1) * BPC)
            m = dict(consts)
            pf = np.stack(
                [phi1[sl], phi2[sl]], axis=0
            )  # [2, BPC, H, W] field-major
            m["pf"] = pf.reshape(2 * BPC * H, W).astype(np_dt)
            in_maps.append(m)
        res = run_bass_kernel_spmd(
            nc, in_maps, core_ids=list(range(N_CORES)), trace=TRACE
        )
        global LAST_EXEC_NS
        LAST_EXEC_NS = res.exec_time_ns
        for r in res.results:
            a = r["acc"].astype(np.float64)
            both = float(a.sum())
            pot += 0.0
            der += both  # pot+der combined in one accumulator

    if host_cells.any():
        nx, ny = _normals(H, W)
        hp, hd = _host_contrib(np.nonzero(host_cells), phi1, phi2, nx, ny)
        pot += hp
        der += hd

    denom = B * n_mask
    return np.float32(WEIGHT * (pot + der) / denom)
